# revision 1
# baseline (speedup 1.0000x reference)
"""Trainium2 Bass kernel: BFP (block-floating-point) activation quantization.

Reference semantics (input NCHW [32, 256, 56, 56] f32):
  per (batch, pixel), channels grouped in blocks of 32:
    maxabs = max |x| over the block
    e      = floor(log2(maxabs))          (guard zero blocks)
    s      = 2^(e-4)                      (5-bit mantissa, QMAX = 31)
    out    = clip(round_half_even(x / s), -31, 31) * s    (0 if maxabs == 0)

Implementation (bit-exact in fp32, validated against the reference):
  s0 = 2^e is extracted by masking the exponent bits of maxabs.  The whole
  round+clip+rescale collapses into one fused DVE op using magic-number
  rounding in the C = 1.5*2^23 * s domain:
      C  = s0 * 786432.0        (= 1.5*2^23 * 2^-4 * s0 = magic * s)
      m  = s0 * 1.9375          (= 31 * s)
      out = min(max(x + C, C - m), C + m) - C
  Every step is exact in fp32: the x + C addition performs the
  round-half-even at ULP = s, the clip bounds and the final subtraction are
  exact multiples of s in the same binade.  The outputs are +-q * 2^(e-4)
  with q <= 31 (5 significant bits), so they are exactly representable in
  bf16 — the backward transposes run in bf16 at half cost.

Layout: channels live on SBUF partitions after the natural NCHW DMA, but the
block reduction needs channels along the free dim, so tiles are transposed
through the (otherwise idle) tensor engine in 128x128 chunks, processed in
the pixel-on-partition layout, and transposed back.  The emission is
software-pipelined (forward transposes run one tile ahead) so the in-order
PE queue never head-of-line blocks on a tile's backward transposes, and
DMAs are split per tile with loads on the SP queue and stores on the ACT
queue so they overlap compute instead of bracketing it.

Sharding: batch 32 -> 4 per core across 8 NeuronCores; no cross-core comms.
"""

import numpy as np

import concourse.bass as bass
import concourse.mybir as mybir
from concourse import bacc, masks, tile
from concourse.bass_utils import run_bass_kernel_spmd

F32 = mybir.dt.float32
BF16 = mybir.dt.bfloat16
I32 = mybir.dt.int32

# ---------------------------------------------------------------------------
# Custom DVE op: the entire quantize in one 1x pass.
#   out = min(max(Src0 + Src1*C0, Src1*C0 - Src1*C1), Src1*C0 + Src1*C1) - Src1*C0
#   Src0 = x (pixel-major tile), Src1 = s0 = 2^e broadcast over the 32-chunk,
#   C0 = 786432.0, C1 = 1.9375.
# ---------------------------------------------------------------------------
_OP_NAME = "BFP_Q5_ANT"


def _bfp_q5_reference(in0, in1, s0, s1, imm2):
    in0 = np.asarray(in0, np.float32)
    in1 = np.asarray(in1, np.float32).reshape(in0.shape)
    c = (in1 * np.float32(s0)).astype(np.float32)
    m = (in1 * np.float32(s1)).astype(np.float32)
    u = (in0 + c).astype(np.float32)
    v = np.minimum(np.maximum(u, (c - m).astype(np.float32)),
                   (c + m).astype(np.float32)).astype(np.float32)
    return (v - c).astype(np.float32)


def _register_custom_op():
    import concourse.dve_ops as dve_ops
    from concourse.dve_ops import DveOp
    from concourse.dve_spec import C0, C1, Spec, Src0, Src1, lower, maxx, minn
    from concourse.dve_uop import DveOpSpec

    for op in dve_ops.OPS:
        if op.name == _OP_NAME:
            return op

    m1 = Src1 * C0
    m2 = Src1 * C1
    spec = Spec(
        body=minn(maxx(Src0 + m1, m1 - m2), m1 + m2) - m1,
        reference=_bfp_q5_reference,
    )
    row = dve_ops._CUSTOM_DVE_ROW_BASE + len(dve_ops.OPS)
    shas = {
        ver: DveOpSpec(
            name=_OP_NAME, opcode=row, uops=lower(spec, ver=ver), rd1_en=True
        ).sha(ver)
        for ver in ("v3", "v4")
    }
    op = DveOp(_OP_NAME, spec, subdim=False, uops_sha=shas)
    dve_ops.OPS.append(op)
    dve_ops.CUSTOM_DVE_SPECS[_OP_NAME] = spec
    dve_ops._SUB_OPCODE_FOR_NAME[_OP_NAME] = row
    return op


# ---------------------------------------------------------------------------
# Tile kernel (per core): x [4, 256, 3136] f32 -> y [4, 256, 3136] f32
# ---------------------------------------------------------------------------
B_PER_CORE = 4
C_CH = 256
HW = 3136          # 56*56 = N_BIG*PX_BIG + 64
PX_BIG = 512
N_BIG = 6
PX_REM = HW - N_BIG * PX_BIG   # 64
N_T = N_BIG + 1
N_C2 = PX_BIG // 128           # 128px chunks per tile
FD = N_C2 * 256                # xt free size
NJ = FD // 32


def bfp_tile_kernel(ctx, tc, y_ap, x_ap):
    nc = tc.nc
    op = _register_custom_op()

    const_pool = ctx.enter_context(tc.tile_pool(name="const", bufs=1))
    x_pool = ctx.enter_context(tc.tile_pool(name="xin", bufs=2))
    o_pool = ctx.enter_context(tc.tile_pool(name="osb", bufs=2))
    xt_pool = ctx.enter_context(tc.tile_pool(name="xt", bufs=3, space="PSUM"))
    on_pool = ctx.enter_context(tc.tile_pool(name="on", bufs=2, space="PSUM"))
    q_pool = ctx.enter_context(tc.tile_pool(name="q", bufs=3))
    m_pool = ctx.enter_context(tc.tile_pool(name="m", bufs=4))

    state = {}
    _idents = {}

    def ensure_idents():
        if _idents:
            return
        ident = const_pool.tile([128, 128], F32, name="ident")
        masks.make_identity(nc, ident[:])
        ident_bf = const_pool.tile([128, 128], BF16, name="ident_bf")
        masks.make_identity(nc, ident_bf[:])
        _idents["f32"] = ident
        _idents["bf16"] = ident_bf

    def emit_fwd(b, px0, npx, x_sb, out_sb):
        """Forward PE transposes of tile (b, px0..px0+npx) into PSUM."""
        if npx >= 128:
            nc2 = npx // 128
            xt = xt_pool.tile([128, nc2 * 256], F32, tag="xt", name=f"xt_{b}_{px0}")
            for c2 in range(nc2):
                for h in range(2):
                    seg = (c2 * 2 + h) * 128
                    nc.tensor.matmul(
                        xt[:, seg:seg + 128],
                        x_sb[:, h, px0 + 128 * c2:px0 + 128 * c2 + 128],
                        _idents["f32"][:, :],
                        is_transpose=True,
                    )
        else:
            xt = xt_pool.tile([64, 256], F32, tag="xt", name=f"xt_{b}_{px0}")
            for h in range(2):
                nc.tensor.matmul(
                    xt[:, h * 128:h * 128 + 128],
                    x_sb[:, h, px0:px0 + npx],
                    _idents["f32"][:, :],
                    is_transpose=True,
                )
        state[(b, px0)] = (xt, npx, out_sb)

    def emit_tail(b, px0):
        """Reduce + quantize + backward transposes + copy-out + store."""
        xt, npx, out_sb = state.pop((b, px0))
        big = npx >= 128
        parts = 128 if big else 64
        fd = xt.shape[1]
        nj = fd // 32
        xt3 = xt[:].rearrange("p (j k) -> p j k", k=32)

        mm = m_pool.tile([parts, nj], F32, tag="m" if big else "ms",
                         name=f"mm_{b}_{px0}")
        # split per PSUM bank (512 f32 cols) so each piece starts as soon as
        # its half of the forward transposes lands
        for lo in range(0, fd, 512):
            hi = min(lo + 512, fd)
            nc.vector.tensor_reduce(
                out=mm[:, lo // 32:hi // 32],
                in_=xt[:, lo:hi].rearrange("p (j k) -> p j k", k=32),
                axis=mybir.AxisListType.X,
                op=mybir.AluOpType.max, apply_absolute_value=True,
            )
        s0 = m_pool.tile([parts, nj], F32, tag="s0" if big else "s0s",
                         name=f"s0_{b}_{px0}")
        nc.vector.tensor_scalar(
            out=s0[:].bitcast(I32), in0=mm[:].bitcast(I32),
            scalar1=23, scalar2=23,
            op0=mybir.AluOpType.logical_shift_right,
            op1=mybir.AluOpType.logical_shift_left,
        )
        q = q_pool.tile([parts, nj * 32], BF16, tag="q", name=f"q_{b}_{px0}")
        nc.vector._custom_dve(
            op,
            out=q[:].rearrange("p (j k) -> p j k", k=32),
            in0=xt3,
            in1=s0[:].unsqueeze(-1).broadcast_to([parts, nj, 32]),
            s0=786432.0, s1=1.9375,
        )

        if big:
            nc2 = npx // 128
            on = on_pool.tile([128, fd], BF16, tag="on", name=f"on_{b}_{px0}")
            for c2 in range(nc2):
                for h in range(2):
                    seg = (c2 * 2 + h) * 128
                    nc.tensor.matmul(
                        on[:, seg:seg + 128],
                        q[:, 256 * c2 + 128 * h:256 * c2 + 128 * h + 128],
                        _idents["bf16"][:, :],
                        is_transpose=True,
                    )
            dst = out_sb[:, :, px0:px0 + npx].rearrange(
                "p h (c k) -> p c h k", k=128)
            nc.scalar.activation(dst, on[:], mybir.ActivationFunctionType.Copy)
        else:
            on = on_pool.tile([128, 128], BF16, tag="on", name=f"on_{b}_{px0}")
            for h in range(2):
                nc.tensor.matmul(
                    on[:, h * npx:(h + 1) * npx],
                    q[:, h * 128:h * 128 + 128],
                    _idents["bf16"][:64, :64],
                    is_transpose=True,
                )
            nc.scalar.activation(
                out_sb[:, :, px0:px0 + npx], on[:],
                mybir.ActivationFunctionType.Copy,
            )
        nc.scalar.dma_start(
            out=y_ap[b].rearrange("(h p) w -> p h w", p=128)[
                :, :, px0:px0 + npx],
            in_=out_sb[:, :, px0:px0 + npx],
        )

    # Software-pipelined emission: fwd transposes run ahead of each tile's
    # tail so the in-order PE queue interleaves them, input chunks are DMA'd
    # per tile with a lead, and batch 0 ramps in with small tiles so the
    # first chain starts as early as possible.
    full = [PX_BIG] * N_BIG + [PX_REM]
    jobs = []
    for b in range(B_PER_CORE):
        px0 = 0
        for npx in full:
            jobs.append((b, px0, npx))
            px0 += npx
    x_sbs, out_sbs = {}, {}

    def emit_in_chunk(b, px0, npx):
        if px0 == 0:
            x_sbs[b] = x_pool.tile([128, 2, HW], F32, tag="x", name=f"x_sb{b}")
            out_sbs[b] = o_pool.tile([128, 2, HW], F32, tag="o", name=f"out_sb{b}")
        xr = x_ap[b].rearrange("(h p) w -> p h w", p=128)
        nc.sync.dma_start(out=x_sbs[b][:, :, px0:px0 + npx],
                          in_=xr[:, :, px0:px0 + npx])

    prefetch = 0
    LAG = 2
    ensure_idents()
    for i, (b, px0, npx) in enumerate(jobs):
        while prefetch < len(jobs) and prefetch <= i + 2:
            emit_in_chunk(*jobs[prefetch])
            prefetch += 1
        emit_fwd(b, px0, npx, x_sbs[b], out_sbs[b])
        if i >= LAG:
            emit_tail(*jobs[i - LAG][:2])
    for j in jobs[len(jobs) - LAG:]:
        emit_tail(*j[:2])


# ---------------------------------------------------------------------------
# Build + run
# ---------------------------------------------------------------------------
_CACHED = {}


def build_bass(n_cores=8):
    from contextlib import ExitStack

    nc = bacc.Bacc(
        "TRN2",
        target_bir_lowering=False,
        debug=False,
        enable_asserts=False,
        num_devices=n_cores,
    )
    x = nc.dram_tensor("activations", [B_PER_CORE, C_CH, HW], F32,
                       kind="ExternalInput").ap()
    y = nc.dram_tensor("out", [B_PER_CORE, C_CH, HW], F32,
                       kind="ExternalOutput").ap()
    with tile.TileContext(nc) as tc:
        with ExitStack() as ctx:
            bfp_tile_kernel(ctx, tc, y, x)
    nc.compile()
    return nc


def kernel(activations: np.ndarray) -> np.ndarray:
    x = np.ascontiguousarray(np.asarray(activations), dtype=np.float32)
    B, C, H, W = x.shape            # [32, 256, 56, 56]
    n_cores = 8
    bpc = B // n_cores              # 4
    xs = x.reshape(n_cores, bpc, C, H * W)
    in_maps = [{"activations": np.ascontiguousarray(xs[c])} for c in range(n_cores)]

    if "nc" not in _CACHED:
        _CACHED["nc"] = build_bass(n_cores)
    nc = _CACHED["nc"]

    res = run_bass_kernel_spmd(nc, in_maps, core_ids=list(range(n_cores)))
    out = np.stack([res.results[c]["out"] for c in range(n_cores)])
    return out.reshape(B, C, H, W).astype(np.float32, copy=False)



# revision 2
# speedup vs baseline: 1.2912x; 1.2912x over previous
"""Trainium2 Bass kernel: BFP (block-floating-point) activation quantization.

Reference semantics (input NCHW [32, 256, 56, 56] f32):
  per (batch, pixel), channels grouped in blocks of 32:
    maxabs = max |x| over the block
    e      = floor(log2(maxabs))          (guard zero blocks)
    s      = 2^(e-4)                      (5-bit mantissa, QMAX = 31)
    out    = clip(round_half_even(x / s), -31, 31) * s    (0 if maxabs == 0)

Implementation (bit-exact in fp32, validated against the reference):
  s0 = 2^e is extracted by masking the exponent bits of maxabs.  The whole
  round+clip+rescale collapses into one fused DVE op using magic-number
  rounding in the C = 1.5*2^23 * s domain:
      C  = s0 * 786432.0        (= 1.5*2^23 * 2^-4 * s0 = magic * s)
      m  = s0 * 1.9375          (= 31 * s)
      out = min(max(x + C, C - m), C + m) - C
  Every step is exact in fp32, and the outputs are +-q * 2^(e-4) with q <= 31
  (5 significant bits), so they are exactly representable in bf16 — the
  kernel stores bf16 and the host widens to f32 (lossless), halving the
  store-side HBM traffic.

Layout: everything runs in the natural NCHW layout (channels on SBUF
partitions).  The cross-partition block-of-32 reduction uses the DVE's
32x32 stream-transpose front-end twice:
  1. tensor_reduce(apply_transpose=True) on x [128, HW/32, 32] reduces the
     transposed 32x32 blocks along X, i.e. across the 32 partitions of each
     channel block: mm[32P+i, g] = max_j |x[32P+j, 32g+i]|.
  2. The fused quantize op runs with transpose_mode=TRANSPOSE on SRC_0: the
     per-block scale s0c (small, [128, HW/32]) is streamed through the same
     front-end with a stride-0 inner broadcast, which lands scale(block P,
     pixel f) on every lane of block P at stream position f — aligned with
     SRC_1 = x streamed naturally.  Output writes bf16 in natural layout.
No tensor-engine transposes, no PSUM, no scalar-engine copies: two DVE
passes over the data + DMA, which leaves the kernel HBM-bound.

Sharding: batch 32 -> 4 per core across 8 NeuronCores; no cross-core comms.
"""

import numpy as np

import concourse.bass as bass
import concourse.mybir as mybir
from concourse import bacc, tile
from concourse.bass_utils import run_bass_kernel_spmd

F32 = mybir.dt.float32
BF16 = mybir.dt.bfloat16
I32 = mybir.dt.int32

# ---------------------------------------------------------------------------
# Custom DVE op: fused quantize with the 32x32 transpose front-end on SRC_0.
#   body: m1 = Src0*C0; m2 = Src0*C1
#         out = min(max(Src1 + m1, m1 - m2), m1 + m2) - m1
#   SRC_0 = s0c broadcast AP [128, G, 32] (stride-0 innermost) -> transposed
#           by the reorder array into per-block scales aligned with SRC_1.
#   SRC_1 = x natural [128, F] f32;  C0 = 786432.0, C1 = 1.9375.
# ---------------------------------------------------------------------------
_OP_NAME = "BFP_Q5T_ANT"


def _bfp_q5t_reference(in0, in1, s0, s1, imm2):
    # Models the hardware: SRC_0's element stream passes through the 32x32
    # transpose reorder array before the ALU body; SRC_1 streams naturally.
    p = in0.shape[0]
    a = np.asarray(in0, np.float32).reshape(p, -1)
    x = np.asarray(in1, np.float32).reshape(p, -1)
    a4 = a.reshape(p // 32, 32, a.shape[1] // 32, 32)
    t = np.ascontiguousarray(a4.transpose(0, 3, 2, 1)).reshape(p, -1)
    c = (t * np.float32(s0)).astype(np.float32)
    m = (t * np.float32(s1)).astype(np.float32)
    u = (x + c).astype(np.float32)
    v = np.minimum(np.maximum(u, (c - m).astype(np.float32)),
                   (c + m).astype(np.float32)).astype(np.float32)
    return (v - c).astype(np.float32)


def _register_custom_op():
    import concourse.dve_ops as dve_ops
    from concourse.dve_ops import DveOp, _COMPILE_CACHE
    from concourse.dve_spec import C0, C1, Spec, Src0, Src1, lower, maxx, minn
    from concourse.dve_uop import DveOpSpec, OpConfig, TransposeMode

    for op in dve_ops.OPS:
        if op.name == _OP_NAME:
            return op

    m1 = Src0 * C0
    m2 = Src0 * C1
    spec = Spec(
        body=minn(maxx(Src1 + m1, m1 - m2), m1 + m2) - m1,
        reference=_bfp_q5t_reference,
    )
    row = dve_ops._CUSTOM_DVE_ROW_BASE + len(dve_ops.OPS)
    ocfg = OpConfig(transpose_mode=TransposeMode.TRANSPOSE)
    shas = {}
    compiled = {}
    for ver in ("v3", "v4"):
        s = DveOpSpec(
            name=_OP_NAME, opcode=row, uops=lower(spec, ver=ver),
            rd1_en=True, op=ocfg,
        )
        s.validate(ver)
        compiled[ver] = s
        shas[ver] = s.sha(ver)
    op = DveOp(_OP_NAME, spec, subdim=False, uops_sha=shas)
    dve_ops.OPS.append(op)
    dve_ops.CUSTOM_DVE_SPECS[_OP_NAME] = spec
    dve_ops._SUB_OPCODE_FOR_NAME[_OP_NAME] = row
    # compile() consults this cache first; seeding it carries the OpConfig
    # (transpose_mode) into the per-NEFF DVE table.
    for ver, s in compiled.items():
        _COMPILE_CACHE[(_OP_NAME, ver)] = s
    return op


# ---------------------------------------------------------------------------
# Tile kernel (per core): x [4, 256, 3136] f32 -> y [4, 256, 3136] bf16
# ---------------------------------------------------------------------------
B_PER_CORE = 4
C_CH = 256
HW = 3136          # 56*56
NG = HW // 32      # 98 channel-block scales per pixel row


def bfp_tile_kernel(ctx, tc, y_ap, x_ap):
    nc = tc.nc
    op = _register_custom_op()

    x_pool = ctx.enter_context(tc.tile_pool(name="xin", bufs=3))
    q_pool = ctx.enter_context(tc.tile_pool(name="qsb", bufs=3))
    m_pool = ctx.enter_context(tc.tile_pool(name="msb", bufs=4))

    jobs = [(b, h) for b in range(B_PER_CORE) for h in range(2)]
    x_sbs = {}

    def emit_load(b, h):
        x_sb = x_pool.tile([128, HW], F32, tag="x", name=f"x_{b}_{h}")
        nc.sync.dma_start(out=x_sb[:], in_=x_ap[b, h * 128:(h + 1) * 128, :])
        x_sbs[(b, h)] = x_sb

    prefetch = 0
    for i, (b, h) in enumerate(jobs):
        while prefetch < len(jobs) and prefetch <= i + 2:
            emit_load(*jobs[prefetch])
            prefetch += 1
        x_sb = x_sbs.pop((b, h))

        mm = m_pool.tile([128, NG], F32, tag="mm", name=f"mm_{b}_{h}")
        nc.vector.tensor_reduce(
            out=mm[:],
            in_=x_sb[:].rearrange("p (g k) -> p g k", k=32),
            axis=mybir.AxisListType.X,
            op=mybir.AluOpType.max,
            apply_absolute_value=True,
            apply_transpose=True,
        )
        s0c = m_pool.tile([128, NG], F32, tag="s0", name=f"s0_{b}_{h}")
        nc.vector.tensor_scalar(
            out=s0c[:].bitcast(I32), in0=mm[:].bitcast(I32),
            scalar1=23, scalar2=23,
            op0=mybir.AluOpType.logical_shift_right,
            op1=mybir.AluOpType.logical_shift_left,
        )
        q = q_pool.tile([128, HW], BF16, tag="q", name=f"q_{b}_{h}")
        nc.vector._custom_dve(
            op,
            out=q[:],
            in0=s0c[:].unsqueeze(-1).broadcast_to([128, NG, 32]),
            in1=x_sb[:],
            s0=786432.0, s1=1.9375,
        )
        nc.scalar.dma_start(out=y_ap[b, h * 128:(h + 1) * 128, :], in_=q[:])


# ---------------------------------------------------------------------------
# Build + run
# ---------------------------------------------------------------------------
_CACHED = {}


def build_bass(n_cores=8):
    from contextlib import ExitStack

    nc = bacc.Bacc(
        "TRN2",
        target_bir_lowering=False,
        debug=False,
        enable_asserts=False,
        num_devices=n_cores,
    )
    x = nc.dram_tensor("activations", [B_PER_CORE, C_CH, HW], F32,
                       kind="ExternalInput").ap()
    y = nc.dram_tensor("out", [B_PER_CORE, C_CH, HW], BF16,
                       kind="ExternalOutput").ap()
    with tile.TileContext(nc) as tc:
        with ExitStack() as ctx:
            bfp_tile_kernel(ctx, tc, y, x)
    nc.compile()
    return nc


def kernel(activations: np.ndarray) -> np.ndarray:
    x = np.ascontiguousarray(np.asarray(activations), dtype=np.float32)
    B, C, H, W = x.shape            # [32, 256, 56, 56]
    n_cores = 8
    bpc = B // n_cores              # 4
    xs = x.reshape(n_cores, bpc, C, H * W)
    in_maps = [{"activations": np.ascontiguousarray(xs[c])} for c in range(n_cores)]

    if "nc" not in _CACHED:
        _CACHED["nc"] = build_bass(n_cores)
    nc = _CACHED["nc"]

    res = run_bass_kernel_spmd(nc, in_maps, core_ids=list(range(n_cores)))
    out = np.stack([np.asarray(res.results[c]["out"]) for c in range(n_cores)])
    return out.reshape(B, C, H, W).astype(np.float32)


# revision 3
# speedup vs baseline: 1.4123x; 1.0938x over previous
"""Trainium2 Bass kernel: BFP (block-floating-point) activation quantization.

Reference semantics (input NCHW [32, 256, 56, 56] f32):
  per (batch, pixel), channels grouped in blocks of 32:
    maxabs = max |x| over the block
    e      = floor(log2(maxabs))          (guard zero blocks)
    s      = 2^(e-4)                      (5-bit mantissa, QMAX = 31)
    out    = clip(round_half_even(x / s), -31, 31) * s    (0 if maxabs == 0)

Implementation (bit-exact in fp32, validated against the reference):
  The whole mask+clip+round+rescale runs as ONE fused DVE op per element:
      e   = maxabs & 0x7F800000          (= 2^floor(log2(maxabs)) as f32)
      m   = e * 1.9375                   (= 31 * s,  s = 2^(e-4))
      y   = min(max(x, 0 - m), m)        (clip first — proven equal to the
                                          reference's round-then-clip at all
                                          half-even boundaries)
      C   = e * 786432.0                 (= 1.5*2^23 * s magic constant)
      out = (y + C) - C                  (round-half-even to a multiple of s)
  Every step is exact in fp32; outputs are +-q * 2^(e-4), q <= 31 (5
  significant bits), hence exactly representable in bf16 — the kernel stores
  bf16 and the host widens to f32 losslessly, halving store HBM traffic.
  The AND mask comes in through the per-partition constant slot as +inf
  (bit pattern 0x7F800000), memset as an integer to dodge non-finite float
  immediates in BIR serialization.

Layout: everything runs in the natural NCHW layout (channels on SBUF
partitions).  The cross-partition block-of-32 reduction uses the DVE's
32x32 stream-transpose front-end twice:
  1. tensor_reduce(apply_transpose=True) on x [128, HW/32, 32] reduces the
     transposed 32x32 blocks along X, i.e. across the 32 partitions of each
     channel block: mm[32P+i, g] = max_j |x[32P+j, 32g+i]|.
  2. The fused quantize op runs with transpose_mode=TRANSPOSE on SRC_0: the
     raw block maxes mm (small, [128, HW/32]) stream through the same
     front-end with a stride-0 inner broadcast, which lands maxabs(block P,
     pixel f) on every lane of block P at stream position f — aligned with
     SRC_1 = x streamed naturally.  Output writes bf16 in natural layout.
No tensor-engine transposes, no PSUM, no scalar-engine copies: two DVE
passes over the data + DMA.  The DVE runs at ~1 elem/lane/cycle (0.96 GHz),
so the kernel sits right at the two-pass DVE / HBM boundary.

Sharding: batch 32 -> 4 per core across 8 NeuronCores; no cross-core comms.
"""

import numpy as np

import concourse.bass as bass
import concourse.mybir as mybir
from concourse import bacc, tile
from concourse.bass_utils import run_bass_kernel_spmd

F32 = mybir.dt.float32
BF16 = mybir.dt.bfloat16
I32 = mybir.dt.int32

_OP_NAME = "BFP_Q5F_ANT"
_EXP_MASK = 0x7F800000


def _bfp_q5f_reference(in0, in1, s0, s1, imm2):
    # Models the hardware: SRC_0's element stream passes through the 32x32
    # transpose reorder array before the ALU body; SRC_1 streams naturally.
    # s0 arrives as the per-partition constant (+inf = the exponent mask),
    # s1 = 786432.0 (magic), imm2 = 1.9375 (clip scale).
    p = in0.shape[0]
    a = np.asarray(in0, np.float32).reshape(p, -1)
    x = np.asarray(in1, np.float32).reshape(p, -1)
    a4 = a.reshape(p // 32, 32, a.shape[1] // 32, 32)
    t = np.ascontiguousarray(a4.transpose(0, 3, 2, 1)).reshape(p, -1)
    mask = np.asarray(s0, np.float32).reshape(-1, 1).view(np.int32)
    e = (t.view(np.int32) & mask).view(np.float32)
    m = (e * np.float32(imm2)).astype(np.float32)
    c = (e * np.float32(s1)).astype(np.float32)
    y = np.minimum(np.maximum(x, (np.float32(0.0) - m).astype(np.float32)), m)
    return ((y + c).astype(np.float32) - c).astype(np.float32)


def _register_custom_op():
    import concourse.dve_ops as dve_ops
    from concourse.dve_ops import DveOp, _COMPILE_CACHE
    from concourse.dve_spec import (
        C0, C1, C2, Bin, Spec, Src0, Src1, Zero, lower, maxx, minn,
    )
    from concourse.dve_uop import AluOp, DveOpSpec, OpConfig, TransposeMode

    for op in dve_ops.OPS:
        if op.name == _OP_NAME:
            return op

    e = Bin(AluOp.BITWISE_AND, Src0, C0)   # C0 = +inf (exp mask), per-partition
    m = e * C2                             # C2 = imm2 = 1.9375 -> 31*s
    y = minn(maxx(Src1, Zero - m), m)      # clip
    c = e * C1                             # C1 = 786432.0 -> magic
    spec = Spec(
        body=(y + c) - c,
        reference=_bfp_q5f_reference,
    )
    row = dve_ops._CUSTOM_DVE_ROW_BASE + len(dve_ops.OPS)
    ocfg = OpConfig(transpose_mode=TransposeMode.TRANSPOSE)
    shas = {}
    compiled = {}
    for ver in ("v3", "v4"):
        s = DveOpSpec(
            name=_OP_NAME, opcode=row, uops=lower(spec, ver=ver),
            rd1_en=True, op=ocfg,
        )
        s.validate(ver)
        compiled[ver] = s
        shas[ver] = s.sha(ver)
    op = DveOp(_OP_NAME, spec, subdim=False, uops_sha=shas)
    dve_ops.OPS.append(op)
    dve_ops.CUSTOM_DVE_SPECS[_OP_NAME] = spec
    dve_ops._SUB_OPCODE_FOR_NAME[_OP_NAME] = row
    # compile() consults this cache first; seeding it carries the OpConfig
    # (transpose_mode) into the per-NEFF DVE table.
    for ver, s in compiled.items():
        _COMPILE_CACHE[(_OP_NAME, ver)] = s
    return op


# ---------------------------------------------------------------------------
# Tile kernel (per core): x [4, 256, 3136] f32 -> y [4, 256, 3136] bf16
# ---------------------------------------------------------------------------
B_PER_CORE = 4
C_CH = 256
HW = 3136          # 56*56


def bfp_tile_kernel(ctx, tc, y_ap, x_ap):
    nc = tc.nc
    op = _register_custom_op()

    const_pool = ctx.enter_context(tc.tile_pool(name="const", bufs=1))
    x_pool = ctx.enter_context(tc.tile_pool(name="xin", bufs=4))
    q_pool = ctx.enter_context(tc.tile_pool(name="qsb", bufs=3))
    m_pool = ctx.enter_context(tc.tile_pool(name="msb", bufs=4))

    c_inf = const_pool.tile([128, 1], F32, name="c_inf")
    nc.gpsimd.memset(c_inf[:].bitcast(I32), _EXP_MASK)

    # (b, h, px0, npx) chunks; first and last (b,h) rows are split in half so
    # the DVE starts on a smaller first load and the last store overlaps the
    # last quantize.
    jobs = []
    for b in range(B_PER_CORE):
        for h in range(2):
            if (b, h) == (0, 0) or (b, h) == (B_PER_CORE - 1, 1):
                jobs.append((b, h, 0, HW // 2))
                jobs.append((b, h, HW // 2, HW // 2))
            else:
                jobs.append((b, h, 0, HW))

    x_sbs = {}

    def emit_load(b, h, px0, npx):
        x_sb = x_pool.tile([128, npx], F32, tag="xh" if npx < HW else "x",
                           name=f"x_{b}_{h}_{px0}")
        nc.sync.dma_start(
            out=x_sb[:],
            in_=x_ap[b, h * 128:(h + 1) * 128, px0:px0 + npx],
        )
        x_sbs[(b, h, px0)] = x_sb

    prefetch = 0
    for i, (b, h, px0, npx) in enumerate(jobs):
        while prefetch < len(jobs) and prefetch <= i + 3:
            emit_load(*jobs[prefetch])
            prefetch += 1
        x_sb = x_sbs.pop((b, h, px0))
        ng = npx // 32

        mm = m_pool.tile([128, ng], F32, tag="mmh" if npx < HW else "mm",
                         name=f"mm_{b}_{h}_{px0}")
        nc.vector.tensor_reduce(
            out=mm[:],
            in_=x_sb[:].rearrange("p (g k) -> p g k", k=32),
            axis=mybir.AxisListType.X,
            op=mybir.AluOpType.max,
            apply_absolute_value=True,
            apply_transpose=True,
        )
        q = q_pool.tile([128, npx], BF16, tag="qh" if npx < HW else "q",
                        name=f"q_{b}_{h}_{px0}")
        nc.vector._custom_dve(
            op,
            out=q[:],
            in0=mm[:].unsqueeze(-1).broadcast_to([128, ng, 32]),
            in1=x_sb[:],
            s0=c_inf[:],
            s1=786432.0,
            imm2=1.9375,
        )
        nc.scalar.dma_start(
            out=y_ap[b, h * 128:(h + 1) * 128, px0:px0 + npx],
            in_=q[:],
        )


# ---------------------------------------------------------------------------
# Build + run
# ---------------------------------------------------------------------------
_CACHED = {}


def build_bass(n_cores=8):
    from contextlib import ExitStack

    nc = bacc.Bacc(
        "TRN2",
        target_bir_lowering=False,
        debug=False,
        enable_asserts=False,
        num_devices=n_cores,
    )
    x = nc.dram_tensor("activations", [B_PER_CORE, C_CH, HW], F32,
                       kind="ExternalInput").ap()
    y = nc.dram_tensor("out", [B_PER_CORE, C_CH, HW], BF16,
                       kind="ExternalOutput").ap()
    with tile.TileContext(nc) as tc:
        with ExitStack() as ctx:
            bfp_tile_kernel(ctx, tc, y, x)
    nc.compile()
    return nc


def kernel(activations: np.ndarray) -> np.ndarray:
    x = np.ascontiguousarray(np.asarray(activations), dtype=np.float32)
    B, C, H, W = x.shape            # [32, 256, 56, 56]
    n_cores = 8
    bpc = B // n_cores              # 4
    xs = x.reshape(n_cores, bpc, C, H * W)
    in_maps = [{"activations": np.ascontiguousarray(xs[c])} for c in range(n_cores)]

    if "nc" not in _CACHED:
        _CACHED["nc"] = build_bass(n_cores)
    nc = _CACHED["nc"]

    res = run_bass_kernel_spmd(nc, in_maps, core_ids=list(range(n_cores)))
    out = np.stack([np.asarray(res.results[c]["out"]) for c in range(n_cores)])
    return out.reshape(B, C, H, W).astype(np.float32)


# revision 5
# speedup vs baseline: 1.4208x; 1.0061x over previous
"""Trainium2 Bass kernel: BFP (block-floating-point) activation quantization.

Reference semantics (input NCHW [32, 256, 56, 56] f32):
  per (batch, pixel), channels grouped in blocks of 32:
    maxabs = max |x| over the block
    e      = floor(log2(maxabs))          (guard zero blocks)
    s      = 2^(e-4)                      (5-bit mantissa, QMAX = 31)
    out    = clip(round_half_even(x / s), -31, 31) * s    (0 if maxabs == 0)

Implementation (bit-exact in fp32, validated against the reference):
  The whole mask+clip+round+rescale runs as ONE fused DVE op per element:
      e   = maxabs & 0x7F800000          (= 2^floor(log2(maxabs)) as f32)
      m   = e * 1.9375                   (= 31 * s,  s = 2^(e-4))
      y   = min(max(x, 0 - m), m)        (clip first — proven equal to the
                                          reference's round-then-clip at all
                                          half-even boundaries)
      C   = e * 786432.0                 (= 1.5*2^23 * s magic constant)
      out = (y + C) - C                  (round-half-even to a multiple of s)
  Every step is exact in fp32; outputs are +-q * 2^(e-4), q <= 31 (5
  significant bits), hence exactly representable in bf16 — the kernel stores
  bf16 and the host widens to f32 losslessly, halving store HBM traffic.
  The AND mask comes in through the per-partition constant slot as +inf
  (bit pattern 0x7F800000), memset as an integer to dodge non-finite float
  immediates in BIR serialization.

Layout: everything runs in the natural NCHW layout (channels on SBUF
partitions).  The cross-partition block-of-32 reduction uses the DVE's
32x32 stream-transpose front-end twice:
  1. tensor_reduce(apply_transpose=True) on x [128, HW/32, 32] reduces the
     transposed 32x32 blocks along X, i.e. across the 32 partitions of each
     channel block: mm[32P+i, g] = max_j |x[32P+j, 32g+i]|.
  2. The fused quantize op runs with transpose_mode=TRANSPOSE on SRC_0: the
     raw block maxes mm (small, [128, HW/32]) stream through the same
     front-end with a stride-0 inner broadcast, which lands maxabs(block P,
     pixel f) on every lane of block P at stream position f — aligned with
     SRC_1 = x streamed naturally.  Output writes bf16 in natural layout.
No tensor-engine transposes, no PSUM, no scalar-engine copies: two DVE
passes over the data + DMA.  The DVE runs at ~1 elem/lane/cycle (0.96 GHz),
so the kernel sits right at the two-pass DVE / HBM boundary.

Sharding: batch 32 -> 4 per core across 8 NeuronCores; no cross-core comms.
"""

import numpy as np

import concourse.bass as bass
import concourse.mybir as mybir
from concourse import bacc, tile
from concourse.bass_utils import run_bass_kernel_spmd

F32 = mybir.dt.float32
BF16 = mybir.dt.bfloat16
I32 = mybir.dt.int32

_OP_NAME = "BFP_Q5F_ANT"
_EXP_MASK = 0x7F800000


def _bfp_q5f_reference(in0, in1, s0, s1, imm2):
    # Models the hardware: SRC_0's element stream passes through the 32x32
    # transpose reorder array before the ALU body; SRC_1 streams naturally.
    # s0 arrives as the per-partition constant (+inf = the exponent mask),
    # s1 = 786432.0 (magic), imm2 = 1.9375 (clip scale).
    p = in0.shape[0]
    a = np.asarray(in0, np.float32).reshape(p, -1)
    x = np.asarray(in1, np.float32).reshape(p, -1)
    a4 = a.reshape(p // 32, 32, a.shape[1] // 32, 32)
    t = np.ascontiguousarray(a4.transpose(0, 3, 2, 1)).reshape(p, -1)
    mask = np.asarray(s0, np.float32).reshape(-1, 1).view(np.int32)
    e = (t.view(np.int32) & mask).view(np.float32)
    m = (e * np.float32(imm2)).astype(np.float32)
    c = (e * np.float32(s1)).astype(np.float32)
    y = np.minimum(np.maximum(x, (np.float32(0.0) - m).astype(np.float32)), m)
    return ((y + c).astype(np.float32) - c).astype(np.float32)


def _register_custom_op():
    import concourse.dve_ops as dve_ops
    from concourse.dve_ops import DveOp, _COMPILE_CACHE
    from concourse.dve_spec import (
        C0, C1, C2, Bin, Spec, Src0, Src1, Zero, lower, maxx, minn,
    )
    from concourse.dve_uop import AluOp, DveOpSpec, OpConfig, TransposeMode

    for op in dve_ops.OPS:
        if op.name == _OP_NAME:
            return op

    e = Bin(AluOp.BITWISE_AND, Src0, C0)   # C0 = +inf (exp mask), per-partition
    m = e * C2                             # C2 = imm2 = 1.9375 -> 31*s
    y = minn(maxx(Src1, Zero - m), m)      # clip
    c = e * C1                             # C1 = 786432.0 -> magic
    spec = Spec(
        body=(y + c) - c,
        reference=_bfp_q5f_reference,
    )
    row = dve_ops._CUSTOM_DVE_ROW_BASE + len(dve_ops.OPS)
    ocfg = OpConfig(transpose_mode=TransposeMode.TRANSPOSE)
    shas = {}
    compiled = {}
    for ver in ("v3", "v4"):
        s = DveOpSpec(
            name=_OP_NAME, opcode=row, uops=lower(spec, ver=ver),
            rd1_en=True, op=ocfg,
        )
        s.validate(ver)
        compiled[ver] = s
        shas[ver] = s.sha(ver)
    op = DveOp(_OP_NAME, spec, subdim=False, uops_sha=shas)
    dve_ops.OPS.append(op)
    dve_ops.CUSTOM_DVE_SPECS[_OP_NAME] = spec
    dve_ops._SUB_OPCODE_FOR_NAME[_OP_NAME] = row
    # compile() consults this cache first; seeding it carries the OpConfig
    # (transpose_mode) into the per-NEFF DVE table.
    for ver, s in compiled.items():
        _COMPILE_CACHE[(_OP_NAME, ver)] = s
    return op


# ---------------------------------------------------------------------------
# Tile kernel (per core): x [4, 256, 3136] f32 -> y [4, 256, 3136] bf16
# ---------------------------------------------------------------------------
B_PER_CORE = 4
C_CH = 256
HW = 3136          # 56*56


def bfp_tile_kernel(ctx, tc, y_ap, x_ap):
    nc = tc.nc
    op = _register_custom_op()

    const_pool = ctx.enter_context(tc.tile_pool(name="const", bufs=1))
    x_pool = ctx.enter_context(tc.tile_pool(name="xin", bufs=5))
    q_pool = ctx.enter_context(tc.tile_pool(name="qsb", bufs=3))
    m_pool = ctx.enter_context(tc.tile_pool(name="msb", bufs=4))

    c_inf = const_pool.tile([128, 1], F32, name="c_inf")
    nc.gpsimd.memset(c_inf[:].bitcast(I32), _EXP_MASK)

    # (b, h, px0, npx) chunks; the first and last (b,h) rows are split into
    # quarters (sizes divisible by 32) so the DVE starts on a small first
    # load and the final stores overlap the last quantizes.
    ramp = [768, 800, 800, 768]
    jobs = []
    for b in range(B_PER_CORE):
        for h in range(2):
            if (b, h) == (0, 0) or (b, h) == (B_PER_CORE - 1, 1):
                px0 = 0
                for npx in ramp:
                    jobs.append((b, h, px0, npx))
                    px0 += npx
            else:
                jobs.append((b, h, 0, HW))

    x_sbs = {}

    def emit_load(b, h, px0, npx):
        x_sb = x_pool.tile([128, npx], F32, tag=f"x{npx}",
                           name=f"x_{b}_{h}_{px0}")
        nc.sync.dma_start(
            out=x_sb[:],
            in_=x_ap[b, h * 128:(h + 1) * 128, px0:px0 + npx],
        )
        x_sbs[(b, h, px0)] = x_sb

    prefetch = 0
    for i, (b, h, px0, npx) in enumerate(jobs):
        while prefetch < len(jobs) and prefetch <= i + 4:
            emit_load(*jobs[prefetch])
            prefetch += 1
        x_sb = x_sbs.pop((b, h, px0))
        ng = npx // 32

        mm = m_pool.tile([128, ng], F32, tag=f"mm{npx}",
                         name=f"mm_{b}_{h}_{px0}")
        nc.vector.tensor_reduce(
            out=mm[:],
            in_=x_sb[:].rearrange("p (g k) -> p g k", k=32),
            axis=mybir.AxisListType.X,
            op=mybir.AluOpType.max,
            apply_absolute_value=True,
            apply_transpose=True,
        )
        q = q_pool.tile([128, npx], BF16, tag=f"q{npx}",
                        name=f"q_{b}_{h}_{px0}")
        nc.vector._custom_dve(
            op,
            out=q[:],
            in0=mm[:].unsqueeze(-1).broadcast_to([128, ng, 32]),
            in1=x_sb[:],
            s0=c_inf[:],
            s1=786432.0,
            imm2=1.9375,
        )
        nc.scalar.dma_start(
            out=y_ap[b, h * 128:(h + 1) * 128, px0:px0 + npx],
            in_=q[:],
        )


# ---------------------------------------------------------------------------
# Build + run
# ---------------------------------------------------------------------------
_CACHED = {}


def build_bass(n_cores=8):
    from contextlib import ExitStack

    nc = bacc.Bacc(
        "TRN2",
        target_bir_lowering=False,
        debug=False,
        enable_asserts=False,
        num_devices=n_cores,
    )
    x = nc.dram_tensor("activations", [B_PER_CORE, C_CH, HW], F32,
                       kind="ExternalInput").ap()
    y = nc.dram_tensor("out", [B_PER_CORE, C_CH, HW], BF16,
                       kind="ExternalOutput").ap()
    with tile.TileContext(nc) as tc:
        with ExitStack() as ctx:
            bfp_tile_kernel(ctx, tc, y, x)
    nc.compile()
    return nc


def kernel(activations: np.ndarray) -> np.ndarray:
    x = np.ascontiguousarray(np.asarray(activations), dtype=np.float32)
    B, C, H, W = x.shape            # [32, 256, 56, 56]
    n_cores = 8
    bpc = B // n_cores              # 4
    xs = x.reshape(n_cores, bpc, C, H * W)
    in_maps = [{"activations": np.ascontiguousarray(xs[c])} for c in range(n_cores)]

    if "nc" not in _CACHED:
        _CACHED["nc"] = build_bass(n_cores)
    nc = _CACHED["nc"]

    res = run_bass_kernel_spmd(nc, in_maps, core_ids=list(range(n_cores)))
    out = np.stack([np.asarray(res.results[c]["out"]) for c in range(n_cores)])
    return out.reshape(B, C, H, W).astype(np.float32)
